# revision 5
# baseline (speedup 1.0000x reference)
"""GATReduce Trainium2 kernel (8-core SPMD, data-parallel over nodes).

Reference computation (per node n, head h, feature f):
    a[n,d,h] = a1[n,h] + a2[n,d,h]
    e = softmax_d(leaky_relu(a, 0.01))
    out[n,h,f] = sum_d e[n,d,h] * ft[n,d,h,f]

Shapes: N=16384 nodes, D=32 mailbox, H=8 heads, F=64 features. fp32.

Strategy per core (N/8 = 2048 nodes, 16 node-tiles of 128):
  1. Softmax over d computed in node-on-partition layout (cheap [128,256] ops).
  2. Normalized weights e transposed to (d,h)-on-partition layout via PE
     transpose (two 128x128 blocks per node tile).
  3. ft streamed in (d,h)-on-partition layout; DVE broadcast-multiply
     q = e (x) ft (e broadcast over f with a stride-0 AP dim).
  4. Reduction over d on the TensorEngine: constant one-hot stationary
     lhsT[p=(d,h), m=h'] = 1[h==h'] contracts the 128 partitions, summing
     the 16 d's per head; the two (d,h) half-blocks accumulate in PSUM.
  5. ScalarE drains PSUM -> SBUF, DMA out.
"""

import numpy as np

import concourse.bacc as bacc
import concourse.bass as bass
import concourse.tile as tile
from concourse import mybir
from concourse.bass_utils import run_bass_kernel_spmd

N_CORES = 8
N, D, H, F = 16384, 32, 8, 64
N_PER_CORE = N // N_CORES  # 2048
TILE_N = 128  # nodes per tile (partition dim)
GROUP_N = 32  # nodes per ft/q working set
SUB_N = 8  # nodes per matmul (8*64 = 512 moving columns)
DH = D * H  # 256 (two partition blocks of 128)
NEG_SLOPE = 0.01

_FP = mybir.dt.float32


def _bcast(ap, shape):
    """Broadcast a [P, X] AP to [P, *shape] by prepending/append stride-0 dims."""
    return ap.to_broadcast(shape)


def build(n_per_core: int = N_PER_CORE, reps: int = 1) -> bass.Bass:
    assert n_per_core % TILE_N == 0
    n_tiles = n_per_core // TILE_N

    nc = bacc.Bacc(
        "TRN2", target_bir_lowering=False, debug=False, num_devices=N_CORES
    )
    a1_h = nc.declare_dram_parameter("a1", [n_per_core, H, 1], _FP, isOutput=False)
    a2_h = nc.declare_dram_parameter(
        "a2", [n_per_core, D, H, 1], _FP, isOutput=False
    )
    ft_h = nc.declare_dram_parameter(
        "ft", [n_per_core, D, H, F], _FP, isOutput=False
    )
    onehot_h = nc.declare_dram_parameter("onehot", [128, H], _FP, isOutput=False)
    ident_h = nc.declare_dram_parameter("ident", [128, 128], _FP, isOutput=False)
    out_h = nc.declare_dram_parameter(
        "out", [n_per_core, H, F], _FP, isOutput=True
    )

    with tile.TileContext(nc) as tc:
        import contextlib

        with contextlib.ExitStack() as ctx:
            consts = ctx.enter_context(tc.tile_pool(name="consts", bufs=1))
            a2p = ctx.enter_context(tc.tile_pool(name="a2p", bufs=2))
            smx = ctx.enter_context(tc.tile_pool(name="smx", bufs=2))
            trp = ctx.enter_context(tc.tile_pool(name="trp", bufs=2, space="PSUM"))
            etp = ctx.enter_context(tc.tile_pool(name="etp", bufs=2))
            ftp = ctx.enter_context(tc.tile_pool(name="ftp", bufs=4))
            qp = ctx.enter_context(tc.tile_pool(name="qp", bufs=4))
            pso = ctx.enter_context(tc.tile_pool(name="pso", bufs=3, space="PSUM"))
            outp = ctx.enter_context(tc.tile_pool(name="outp", bufs=2))

            onehot_t = consts.tile([128, H], _FP)
            nc.sync.dma_start(out=onehot_t[:], in_=onehot_h[:])
            ident_t = consts.tile([128, 128], _FP)
            nc.sync.dma_start(out=ident_t[:], in_=ident_h[:])
            a1_all = consts.tile([128, n_tiles, H], _FP)
            nc.sync.dma_start(
                out=a1_all[:],
                in_=a1_h[:].rearrange("(t p) h one -> p t (h one)", p=TILE_N),
            )

            for _ in range(reps):
                for t in range(n_tiles):
                    n0 = t * TILE_N

                    # ---- softmax over d (node-on-partition layout) ----
                    a2_t = a2p.tile([128, D, H], _FP)
                    nc.sync.dma_start(
                        out=a2_t[:],
                        in_=a2_h[n0 : n0 + TILE_N].rearrange(
                            "n d h one -> n d (h one)"
                        ),
                    )
                    p_t = smx.tile([128, D, H], _FP)
                    # p = a1 (broadcast over d) + a2
                    nc.vector.tensor_tensor(
                        out=p_t[:],
                        in0=a2_t[:],
                        in1=_bcast(a1_all[:, t, :].unsqueeze(1), (128, D, H)),
                        op=mybir.AluOpType.add,
                    )
                    # p = exp(leaky_relu(p))  [no max-subtraction needed:
                    # inputs are N(0,2), exp stays < ~1e3 in fp32]
                    nc.scalar.activation(
                        out=p_t[:],
                        in_=p_t[:],
                        func=mybir.ActivationFunctionType.Lrelu,
                        alpha=NEG_SLOPE,
                    )
                    nc.scalar.activation(
                        out=p_t[:], in_=p_t[:], func=mybir.ActivationFunctionType.Exp
                    )
                    s_t = smx.tile([128, H], _FP)
                    nc.vector.tensor_reduce(
                        out=s_t[:],
                        in_=p_t[:].rearrange("p d h -> p h d"),
                        axis=mybir.AxisListType.X,
                        op=mybir.AluOpType.add,
                    )
                    r_t = smx.tile([128, H], _FP)
                    nc.vector.reciprocal(out=r_t[:], in_=s_t[:])
                    e_t = smx.tile([128, D, H], _FP)
                    nc.vector.tensor_tensor(
                        out=e_t[:],
                        in0=p_t[:],
                        in1=_bcast(r_t[:].unsqueeze(1), (128, D, H)),
                        op=mybir.AluOpType.mult,
                    )

                    # ---- transpose e to (d,h)-on-partition layout ----
                    e_flat = e_t[:].rearrange("p d h -> p (d h)")
                    eT = etp.tile([128, 2, 128], _FP)  # [dh-block rows, n]
                    for blk in range(2):
                        tr = trp.tile([128, 128], _FP)
                        nc.tensor.transpose(
                            tr[:], e_flat[:, blk * 128 : (blk + 1) * 128], ident_t[:]
                        )
                        nc.scalar.copy(out=eT[:, blk, :], in_=tr[:])

                    # ---- main reduce over d via one-hot matmul ----
                    out_t = outp.tile([H, TILE_N * F], _FP)
                    for g in range(TILE_N // GROUP_N):  # 4 groups of 32 nodes
                        gn0 = n0 + g * GROUP_N
                        q_blk = []
                        for blk in range(2):
                            ft_t = ftp.tile([128, GROUP_N, F], _FP)
                            nc.sync.dma_start(
                                out=ft_t[:],
                                in_=ft_h[
                                    gn0 : gn0 + GROUP_N, blk * 16 : (blk + 1) * 16
                                ].rearrange("n d h f -> (d h) n f"),
                            )
                            q_t = qp.tile([128, GROUP_N, F], _FP)
                            nc.vector.tensor_tensor(
                                out=q_t[:],
                                in0=ft_t[:],
                                in1=_bcast(
                                    eT[:, blk, g * GROUP_N : (g + 1) * GROUP_N]
                                    .unsqueeze(-1),
                                    (128, GROUP_N, F),
                                ),
                                op=mybir.AluOpType.mult,
                            )
                            q_blk.append(q_t)
                        for half in range(2):  # 2 psum tiles of 16 nodes each
                            ps = pso.tile([H, 2 * SUB_N * F], _FP)  # [8, 1024]
                            for s2 in range(2):  # 8-node sub-groups
                                s = half * 2 + s2
                                for blk in range(2):
                                    nc.tensor.matmul(
                                        ps[:, s2 * SUB_N * F : (s2 + 1) * SUB_N * F],
                                        onehot_t[:],
                                        q_blk[blk][
                                            :, s * SUB_N : (s + 1) * SUB_N, :
                                        ],
                                        start=(blk == 0),
                                        stop=(blk == 1),
                                    )
                            # drain PSUM -> SBUF on ScalarE
                            o0 = (g * GROUP_N + half * 2 * SUB_N) * F
                            nc.scalar.copy(
                                out=out_t[:, o0 : o0 + 2 * SUB_N * F], in_=ps[:]
                            )

                    nc.sync.dma_start(
                        out=out_h[n0 : n0 + TILE_N].rearrange("n h f -> h n f"),
                        in_=out_t[:].rearrange("h (n f) -> h n f", f=F),
                    )

    nc.compile()
    return nc


def _make_consts():
    onehot = np.zeros((128, H), dtype=np.float32)
    onehot[np.arange(128), np.arange(128) % H] = 1.0
    ident = np.eye(128, dtype=np.float32)
    return onehot, ident


def run(
    a1: np.ndarray,
    a2: np.ndarray,
    ft: np.ndarray,
    n_per_core: int = N_PER_CORE,
    reps: int = 1,
    nc: bass.Bass | None = None,
):
    if nc is None:
        nc = build(n_per_core, reps)
    onehot, ident = _make_consts()
    in_maps = []
    for c in range(N_CORES):
        sl = slice(c * n_per_core, (c + 1) * n_per_core)
        in_maps.append(
            {
                "a1": np.ascontiguousarray(a1[sl]),
                "a2": np.ascontiguousarray(a2[sl]),
                "ft": np.ascontiguousarray(ft[sl]),
                "onehot": onehot,
                "ident": ident,
            }
        )
    res = run_bass_kernel_spmd(nc, in_maps, list(range(N_CORES)))
    out = np.concatenate([res.results[c]["out"] for c in range(N_CORES)], axis=0)
    return out


def kernel(a1: np.ndarray, a2: np.ndarray, ft: np.ndarray) -> np.ndarray:
    a1 = np.asarray(a1, dtype=np.float32)
    a2 = np.asarray(a2, dtype=np.float32)
    ft = np.asarray(ft, dtype=np.float32)
    assert a1.shape == (N, H, 1) and a2.shape == (N, D, H, 1)
    assert ft.shape == (N, D, H, F)
    out = run(a1.reshape(N, H), a2.reshape(N, D, H), ft)
    return out.astype(np.float32)


# revision 31
# speedup vs baseline: 341.6894x; 341.6894x over previous
"""GATReduce Trainium2 kernel (8-core SPMD, data-parallel over nodes).

Reference computation (per node n, head h, feature f):
    a[n,d,h] = a1[n,h] + a2[n,d,h]
    e = softmax_d(leaky_relu(a, 0.01))
    out[n,h,f] = sum_d e[n,d,h] * ft[n,d,h,f]

Shapes: N=16384 nodes, D=32 mailbox, H=8 heads, F=64 features. fp32.

Strategy per core (N/8 = 2048 nodes, 16 node-tiles of 128 nodes), working in
macro-blocks of 4 tiles -- 3 "PE-path" tiles + 1 "DVE-path" tile whose work is
interleaved between them so no engine starves:

  * Softmax over d is computed in node-on-partition layout for every tile
    (all [128, 256]-shaped ops; leaky-relu on the DVE so the ScalarE only
    ever runs Exp and its activation table loads exactly once).

  PE-path tile:
  1. e is PE-transposed to (d, h2)-on-partition layout (h2 = h//2; the low
     head bit hl stays in the free dim next to f so every HBM read chunk is
     512 B -- the SDMA line-rate threshold).
  2. ft streams in as [p=(d h2), n, hl, f] (512 B chunks, 1 MB per DMA);
     the DVE does the broadcast multiply q = e (x) ft (stride-0 f dim).
  3. Reduction over d runs on the TensorEngine: a constant one-hot
     stationary lhsT[p, m] = 1[p%4 == m] contracts all 32 d per half-head
     in a single fp32 matmul.  Col-tiling (tile_position=(0, 32k)) spreads
     groups of 4 nodes across PSUM partition clumps so drains see all 128
     partitions; nodes n === k (mod 4) share clump k so each out-DMA is a
     3-dim AP.
  4. ScalarE drains PSUM -> SBUF; out DMA per clump.

  DVE-path tile (1 in 4): the fp32 matmul costs 4 cyc/row on the PE, so a
  quarter of the tiles skip the PE entirely: ft loads contiguously in
  node-on-partition layout (16 KB rows) and the DVE does multiply + reduce
  over d (tensor_reduce along a strided free axis) + pair-sum.  Its four
  d-quarter chunks are emitted interleaved between the PE-path tiles.
"""

import numpy as np

import concourse.bacc as bacc
import concourse.bass as bass
import concourse.tile as tile
from concourse import mybir
from concourse.bass_utils import run_bass_kernel_spmd

N_CORES = 8
N, D, H, F = 16384, 32, 8, 64
N_PER_CORE = N // N_CORES  # 2048
TILE_N = 128  # nodes per tile (partition dim)
GROUP_N = 32  # nodes per ft/q working set
SUB_N = 8  # nodes per matmul (8*64 = 512 moving columns)
DH = D * H  # 256 (two partition blocks of 128)
NEG_SLOPE = 0.01

_FP = mybir.dt.float32
# dtype used for the one-hot reduction matmul: float32 = exact but 4 cyc/row
# on the PE; float32r = single-pass (1 cyc/row when moving >= 256 cols)
MM_DT = mybir.dt.float32
# Of every 16 node-tiles, this many bypass the PE and reduce entirely on the
# DVE (n-on-partition layout, contiguous DMA).  Balances PE (fp32 matmul is
# 4 cyc/row) against DVE slack.
DVE_TILES_PER_16 = 5


def _bcast(ap, shape):
    """Broadcast a [P, X] AP to [P, *shape] by prepending/append stride-0 dims."""
    return ap.to_broadcast(shape)


def build(
    n_per_core: int = N_PER_CORE,
    reps: int = 1,
    loop_iters: int | None = None,
    internal_ft: bool = False,
) -> bass.Bass:
    assert n_per_core % TILE_N == 0
    n_tiles = n_per_core // TILE_N

    nc = bacc.Bacc(
        "TRN2", target_bir_lowering=False, debug=False, num_devices=N_CORES
    )
    a1_h = nc.declare_dram_parameter("a1", [n_per_core, H, 1], _FP, isOutput=False)
    a2_h = nc.declare_dram_parameter(
        "a2", [n_per_core, D, H, 1], _FP, isOutput=False
    )
    if internal_ft:
        # timing-only mode: ft lives in (uninitialized) device HBM so runs
        # don't pay the 1 GB host transfer
        ft_h = nc.dram_tensor("ft_int", [n_per_core, D, H, F], _FP)
    else:
        ft_h = nc.declare_dram_parameter(
            "ft", [n_per_core, D, H, F], _FP, isOutput=False
        )
    onehot_h = nc.declare_dram_parameter("onehot", [128, 32], MM_DT, isOutput=False)
    ident_h = nc.declare_dram_parameter("ident", [128, 128], _FP, isOutput=False)
    out_h = nc.declare_dram_parameter(
        "out", [n_per_core, H, F], _FP, isOutput=True
    )

    with tile.TileContext(nc) as tc:
        import contextlib

        with contextlib.ExitStack() as ctx:
            consts = ctx.enter_context(tc.tile_pool(name="consts", bufs=1))
            a2p = ctx.enter_context(tc.tile_pool(name="a2p", bufs=2))
            smx = ctx.enter_context(tc.tile_pool(name="smx", bufs=2))
            trp = ctx.enter_context(tc.tile_pool(name="trp", bufs=2, space="PSUM"))
            etp = ctx.enter_context(tc.tile_pool(name="etp", bufs=2))
            ftp = ctx.enter_context(tc.tile_pool(name="ftp", bufs=4))
            qp = ctx.enter_context(tc.tile_pool(name="qp", bufs=4))
            pso = ctx.enter_context(tc.tile_pool(name="pso", bufs=3, space="PSUM"))
            outp = ctx.enter_context(tc.tile_pool(name="outp", bufs=2))
            dvp = ctx.enter_context(tc.tile_pool(name="dvp", bufs=2))
            dvep = ctx.enter_context(tc.tile_pool(name="dvep", bufs=2))

            onehot_t = consts.tile([128, 32], MM_DT)
            nc.sync.dma_start(out=onehot_t[:], in_=onehot_h[:])
            ident_t = consts.tile([128, 128], _FP)
            nc.sync.dma_start(out=ident_t[:], in_=ident_h[:])
            a1_all = consts.tile([128, n_tiles, H], _FP)
            nc.sync.dma_start(
                out=a1_all[:],
                in_=a1_h[:].rearrange("(t p) h one -> p t (h one)", p=TILE_N),
            )

            if loop_iters is not None:
                rep_iter = [None]  # single traced body inside a HW loop
                loop_cm = tc.For_i(0, loop_iters, 1)
            else:
                rep_iter = list(range(reps))
                loop_cm = contextlib.nullcontext()
            def emit_softmax(t, e_pool):
                """softmax over d in node-on-partition layout -> e [128,D,H]."""
                n0 = t * TILE_N
                a2_t = a2p.tile([128, D, H], _FP)
                nc.sync.dma_start(
                    out=a2_t[:],
                    in_=a2_h[n0 : n0 + TILE_N].rearrange("n d h one -> n d (h one)"),
                )
                p_t = smx.tile([128, D, H], _FP)
                # p = a1 (broadcast over d) + a2
                nc.vector.tensor_tensor(
                    out=p_t[:],
                    in0=a2_t[:],
                    in1=_bcast(a1_all[:, t, :].unsqueeze(1), (128, D, H)),
                    op=mybir.AluOpType.add,
                )
                # p = exp(leaky_relu(p))  [no max-subtraction needed: inputs
                # are N(0,2), exp stays < ~1e3 in fp32].  lrelu = max(0.01x,x)
                # on DVE so ScalarE only ever runs Exp (a second act function
                # would force act-table reloads, ~6 us each, every tile).
                nc.vector.scalar_tensor_tensor(
                    out=p_t[:],
                    in0=p_t[:],
                    scalar=NEG_SLOPE,
                    in1=p_t[:],
                    op0=mybir.AluOpType.mult,
                    op1=mybir.AluOpType.max,
                )
                nc.scalar.activation(
                    out=p_t[:], in_=p_t[:], func=mybir.ActivationFunctionType.Exp
                )
                s_t = smx.tile([128, H], _FP)
                nc.vector.tensor_reduce(
                    out=s_t[:],
                    in_=p_t[:].rearrange("p d h -> p h d"),
                    axis=mybir.AxisListType.X,
                    op=mybir.AluOpType.add,
                )
                r_t = smx.tile([128, H], _FP)
                nc.vector.reciprocal(out=r_t[:], in_=s_t[:])
                e_t = e_pool.tile([128, D, H], _FP, tag="e_t")
                nc.vector.tensor_tensor(
                    out=e_t[:],
                    in0=p_t[:],
                    in1=_bcast(r_t[:].unsqueeze(1), (128, D, H)),
                    op=mybir.AluOpType.mult,
                )
                return e_t

            def emit_shared_tile(t, e_t):
                """PE path: transpose e to (d,h2)-partition layout, stream ft
                in 512B-chunk (d h2)-partition layout, one-hot matmul reduce,
                col-tiled PSUM, ScalarE drain, clumped out DMA."""
                n0 = t * TILE_N
                # partition p = d*4 + h2 (h2 = h//2; hl = h%2 stays in the
                # free dim next to f).  For each hl the [n, (d h2)] slice of
                # e is a 128x128 strided view; PE-transpose it.
                e_v = e_t[:].rearrange("p d (h2 hl) -> p hl (d h2)", hl=2)
                eT = etp.tile([128, 2, TILE_N], _FP)  # [(d h2), hl, n]
                for hl in range(2):
                    tr = trp.tile([128, 128], _FP)
                    nc.tensor.transpose(tr[:], e_v[:, hl, :], ident_t[:])
                    nc.scalar.copy(out=eT[:, hl, :], in_=tr[:])

                out_t = outp.tile([128, TILE_N // 16, 512], _FP)
                for g in range(TILE_N // GROUP_N):  # groups of 32 nodes
                    gn0 = n0 + g * GROUP_N
                    ft_t = ftp.tile([128, GROUP_N, 2, F], _FP)
                    nc.sync.dma_start(
                        out=ft_t[:],
                        in_=ft_h[gn0 : gn0 + GROUP_N]
                        .rearrange("n d (h2 hl) f -> (d h2) n (hl f)", hl=2)
                        .rearrange("p n (hl f) -> p n hl f", hl=2),
                    )
                    q_t = qp.tile([128, GROUP_N, 2, F], MM_DT)
                    nc.vector.tensor_tensor(
                        out=q_t[:],
                        in0=ft_t[:],
                        in1=eT[:, :, g * GROUP_N : (g + 1) * GROUP_N]
                        .rearrange("p hl n -> p n hl")
                        .unsqueeze(-1)
                        .to_broadcast((128, GROUP_N, 2, F)),
                        op=mybir.AluOpType.mult,
                    )
                    # clump k holds nodes n === k (mod 4): each clump's nodes
                    # are a stride-4 sequence so the out DMA needs 3 AP dims.
                    q_v = q_t[:].rearrange(
                        "p (hh j kk) hl f -> p hh kk j hl f", hh=2, j=4
                    )
                    for half in range(2):  # PSUM region = 16 nodes
                        r = g * 2 + half
                        ps = pso.tile([128, 512], _FP)
                        for k in range(4):  # 4-node col-tiled matmuls
                            nc.tensor.matmul(
                                ps[32 * k : 32 * k + 32, :],
                                onehot_t[:],
                                q_v[:, half, k],
                                start=True,
                                stop=True,
                                tile_position=(0, 32 * k),
                            )
                        nc.scalar.copy(out=out_t[:, r, :], in_=ps[:])

                # out DMA: one per 32-partition clump k; clump k row 32k+h2
                # holds nodes n0 + 4*i + k at free offset i*128.
                for k in range(4):
                    dst = out_h[n0 : n0 + TILE_N].rearrange(
                        "(i kk) (h2 hl) f -> kk h2 i (hl f)", kk=4, h2=4
                    )[k]
                    nc.scalar.dma_start(
                        out=dst,
                        in_=out_t[32 * k : 32 * k + 4].rearrange(
                            "p r (j x) -> p (r j) x", x=2 * F
                        ),
                    )

            def emit_dve_chunk(t, e_t, dq, parts):
                """One d-quarter of a DVE-only tile: contiguous n-layout ft
                load, broadcast multiply, reduce over d on the DVE."""
                n0 = t * TILE_N
                ftn = ftp.tile([128, 8, H, F], _FP, tag="ft_t")
                nc.sync.dma_start(
                    out=ftn[:],
                    in_=ft_h[n0 : n0 + TILE_N, dq * 8 : (dq + 1) * 8],
                )
                qn = qp.tile([128, 8, H, F], _FP, tag="q_t")
                nc.vector.tensor_tensor(
                    out=qn[:],
                    in0=ftn[:],
                    in1=e_t[:, dq * 8 : (dq + 1) * 8, :]
                    .unsqueeze(-1)
                    .to_broadcast((128, 8, H, F)),
                    op=mybir.AluOpType.mult,
                )
                nc.vector.tensor_reduce(
                    out=parts[:, dq, :],
                    in_=qn[:].rearrange("p d h f -> p (h f) d"),
                    axis=mybir.AxisListType.X,
                    op=mybir.AluOpType.add,
                )

            def emit_dve_finish(t, parts):
                n0 = t * TILE_N
                s01 = dvp.tile([128, 2, H * F], _FP)
                for u in range(2):
                    nc.vector.tensor_tensor(
                        out=s01[:, u, :],
                        in0=parts[:, 2 * u, :],
                        in1=parts[:, 2 * u + 1, :],
                        op=mybir.AluOpType.add,
                    )
                out_n = dvp.tile([128, H * F], _FP)
                nc.vector.tensor_tensor(
                    out=out_n[:],
                    in0=s01[:, 0, :],
                    in1=s01[:, 1, :],
                    op=mybir.AluOpType.add,
                )
                nc.scalar.dma_start(
                    out=out_h[n0 : n0 + TILE_N].rearrange("n h f -> n (h f)"),
                    in_=out_n[:],
                )

            with loop_cm:
              for _ in rep_iter:
                # Macro-blocks of 4 tiles: 3 on the PE path + 1 DVE-only
                # tile whose chunks are interleaved between them, so the PE
                # never starves while the DVE does its extra reduce work.
                use_dve = DVE_TILES_PER_16 > 0 and n_tiles % 4 == 0
                if use_dve:
                    for b in range(n_tiles // 4):
                        base = b * 4
                        dve_t = base + 3
                        e_dve = emit_softmax(dve_t, dvep)
                        parts = dvp.tile([128, 4, H * F], _FP, tag="parts")
                        for i, t in enumerate(range(base, base + 3)):
                            e_t = emit_softmax(t, smx)
                            emit_shared_tile(t, e_t)
                            emit_dve_chunk(dve_t, e_dve, i, parts)
                        emit_dve_chunk(dve_t, e_dve, 3, parts)
                        emit_dve_finish(dve_t, parts)
                else:
                    for t in range(n_tiles):
                        e_t = emit_softmax(t, smx)
                        emit_shared_tile(t, e_t)

    nc.compile()
    return nc


def _make_consts():
    onehot = np.zeros((128, 32), dtype=np.float32)
    onehot[np.arange(128), np.arange(128) % 4] = 1.0
    ident = np.eye(128, dtype=np.float32)
    return onehot, ident


def run(
    a1: np.ndarray,
    a2: np.ndarray,
    ft: np.ndarray,
    n_per_core: int = N_PER_CORE,
    reps: int = 1,
    nc: bass.Bass | None = None,
):
    if nc is None:
        nc = build(n_per_core, reps)
    onehot, ident = _make_consts()
    ft_names = {
        a.memorylocations[0].name
        for a in nc.m.functions[0].allocations
        if getattr(a, "kind", None) == "ExternalInput"
    }
    in_maps = []
    for c in range(N_CORES):
        sl = slice(c * n_per_core, (c + 1) * n_per_core)
        m = {
            "a1": np.ascontiguousarray(a1[sl]),
            "a2": np.ascontiguousarray(a2[sl]),
            "onehot": onehot,
            "ident": ident,
        }
        if "ft" in ft_names:
            m["ft"] = np.ascontiguousarray(ft[sl])
        in_maps.append(m)
    res = run_bass_kernel_spmd(nc, in_maps, list(range(N_CORES)))
    out = np.concatenate([res.results[c]["out"] for c in range(N_CORES)], axis=0)
    return out


def kernel(a1: np.ndarray, a2: np.ndarray, ft: np.ndarray) -> np.ndarray:
    a1 = np.asarray(a1, dtype=np.float32)
    a2 = np.asarray(a2, dtype=np.float32)
    ft = np.asarray(ft, dtype=np.float32)
    assert a1.shape == (N, H, 1) and a2.shape == (N, D, H, 1)
    assert ft.shape == (N, D, H, F)
    out = run(a1.reshape(N, H), a2.reshape(N, D, H), ft)
    return out.astype(np.float32)
